# revision 27
# baseline (speedup 1.0000x reference)
"""Expert-parallel MoE SwiGLU kernel for 8 Trainium2 NeuronCores.

Strategy: expert parallelism with host-side dispatch/combine. Each of the
8 cores owns one expert's weights. The host routes tokens by expert_idx,
packs each expert's tokens as a transposed [D, W] panel (features on
partitions so no on-chip transposes are needed anywhere), and each core
runs a dense SwiGLU FFN:  yT = w_down.T-blocks @ (silu(wg.T@xT) * (wu.T@xT)).
Matmul operands stream as fp16 (fp32 PSUM accumulation; ~6e-4 max
relative error vs the fp32 reference), halving the weight traffic that
dominates this memory-bound kernel. fp8 (DoubleRow) was evaluated and
rejected: every quantization site alone (x, w_gate/up, t, w_down)
exceeds the 2e-2 max-relative-error budget (measured 2.8e-2..3.9e-2).

Schedule design, from perfetto-trace supply modeling:
- DMA rings are blocked until the ~7.2us framework preamble ends, then
  HBM sustains ~0.36MB/us/core. Descriptor issue costs ~650ns of
  ring-engine time, and a matmul chain needs its group's whole weight
  set anyway, so weights stream as ONE descriptor per matrix per
  f-group, all on the sync ring -- the only engine queue with no
  compute ops that could block its in-order FIFO (the scalar queue
  stalls behind silu ACTIVATEs waiting on PSUM).
- f-group widths are graduated (128x4, 256x2, 512x6): small groups in
  front so the first gate chain starts ~4us earlier than a uniform
  split allows, wide groups later to amortize issue cost.
- within a group, all gate chains+silus are emitted before up chains,
  matching the gate-before-up arrival order on the wire; the previous
  group's down-projection chains interleave into the up phase.
- each group's w_down descriptor is issued one group late (first use is
  one group later), pulling early gate/up arrivals forward.
- a ~17-matmul dummy burst at the start keeps the PE activity monitor
  fed so the clock is at 2.4GHz (not the cold 1.2GHz) when real work
  lands; without it the HAM revokes full clock for 7-13us.
- the final group's adds write fp16 staging tiles streamed out per
  d-tile on alternating rings, so the output drain after the last
  matmul is ~1us.
- all SBUF tiles are 2D with flat index arithmetic (3D tile slices
  measured ~27ns/matmul slower, though part of that may have been a
  throttled run); tile pools use bufs>=3 -- bufs=2 produced wrong
  results with consecutive small groups (tile lifetime spans 2 groups).

Steady state: matmuls issue every ~121ns (274-col moving panel = 114ns
at 2.4GHz + ~7ns sequencer overhead), PE-bound at the fp16 rate with
zero mid-stream stalls; HW exec ~112us vs the 91us pure-matmul floor
(preamble ~7us, supply-bound fill ~4.5us, per-instruction overhead
~5.5us, output drain + teardown ~4us).
"""

import numpy as np
from contextlib import ExitStack

D_MODEL = 1024
D_FF = 4096
N_EXPERTS = 8
N_CORES = 8

_ND = D_MODEL // 128  # 8 contraction chunks over d_model
_NF = D_FF // 128     # 32 f chunks

# graduated f-group widths (columns); must sum to D_FF
_GROUPS = [128, 128, 128, 128, 256, 256, 512, 512, 512, 512, 512, 512]
assert sum(_GROUPS) == D_FF

_nc_cache = {}

# compute dtype for matmul operands: "float32r" (safest), "float16", "bfloat16"
import os as _os
_CDT = _os.environ.get("MOE_KERNEL_DTYPE", "float16")
_WARM = int(_os.environ.get("MOE_WARMUP", "13"))


def _np_cdt():
    if _CDT == "float16":
        return np.float16
    if _CDT == "bfloat16":
        import ml_dtypes
        return ml_dtypes.bfloat16
    return np.float32


def _build_nc(W: int):
    """Build + schedule the per-core Bass program for token capacity W."""
    import concourse.bacc as bacc
    import concourse.tile as tile
    from concourse import mybir

    f32 = mybir.dt.float32
    f32r = getattr(mybir.dt, _CDT)

    nc = bacc.Bacc("TRN2", target_bir_lowering=False, debug=False,
                   num_devices=N_CORES)
    xt = nc.dram_tensor("xt", [128, _ND * W], f32r, kind="ExternalInput").ap()
    # wg/wu: per-group blocks [128, ND*Fg] concatenated along the free dim
    wg = nc.dram_tensor("wg", [128, _ND * D_FF], f32r,
                        kind="ExternalInput").ap()
    wu = nc.dram_tensor("wu", [128, _ND * D_FF], f32r,
                        kind="ExternalInput").ap()
    # wd: f-tile-major [128, NF*D]: line p holds rows (c*128+p) of w_down
    wd = nc.dram_tensor("wd", [128, _NF * D_MODEL], f32r,
                        kind="ExternalInput").ap()
    yt = nc.dram_tensor("yt", [128, _ND * W], f32r, kind="ExternalOutput").ap()

    with tile.TileContext(nc) as tc, ExitStack() as ctx:
        xpool = ctx.enter_context(tc.tile_pool(name="x", bufs=1))
        wgp = ctx.enter_context(tc.tile_pool(name="wgp", bufs=3))
        wup = ctx.enter_context(tc.tile_pool(name="wup", bufs=3))
        wdp = ctx.enter_context(tc.tile_pool(name="wdp", bufs=3))
        tp = ctx.enter_context(tc.tile_pool(name="tp", bufs=3))
        gap = ctx.enter_context(tc.tile_pool(name="gap", bufs=3))
        yp = ctx.enter_context(tc.tile_pool(name="yp", bufs=1))
        pg = ctx.enter_context(tc.tile_pool(name="pg", bufs=2, space="PSUM"))
        pu = ctx.enter_context(tc.tile_pool(name="pu", bufs=2, space="PSUM"))
        pd = ctx.enter_context(tc.tile_pool(name="pd", bufs=4, space="PSUM"))

        # Input panel, split around the first gate descriptor: the lower
        # d-chunks unblock the first chain's first matmuls one descriptor
        # earlier; the upper half follows behind gate group 0.
        x_t = xpool.tile([128, _ND * W], f32r, tag="x")
        nc.sync.dma_start(x_t[:, 0:4 * W], xt[:, 0:4 * W])

        def x_sl(d):
            return x_t[:, d * W:d * W + W]

        y_acc = [yp.tile([128, W], f32, tag=f"y{d}", name=f"y_acc{d}")
                 for d in range(_ND)]

        scr_w = xpool.tile([128, 128], f32r, tag="scrw", name="scr_w")
        scr_x = xpool.tile([128, W], f32r, tag="scrx", name="scr_x")
        nc.vector.memset(scr_w[:], 0.0)
        nc.vector.memset(scr_x[:], 0.0)
        scr_p = pd.tile([128, W], f32, tag="pd", name="scr_p")
        scr_p2 = pd.tile([128, W], f32, tag="pd", name="scr_p2")
        _scr = [scr_p, scr_p2]

        def emit_warmup(n):
            for i in range(n):
                nc.tensor.matmul(_scr[i % 2][:], scr_w[:], scr_x[:],
                                 start=True, stop=True)

        # opening burst: continuous PE activity bridging the gap between
        # ring unblock and the first weight group's arrival
        emit_warmup(_WARM)

        def emit_down(pg_idx, t_tiles, wd_t, dts, last):
            # y[dt] += wd[fgroup rows, dt cols].T @ t   for dt in dts
            nft = len(t_tiles)
            for dt in dts:
                pdt = pd.tile([128, W], f32, tag="pd", name=f"pd_{pg_idx}_{dt}")
                for ft in range(nft):
                    nc.tensor.matmul(
                        pdt[:],
                        wd_t[:, ft * D_MODEL + dt * 128:
                             ft * D_MODEL + dt * 128 + 128],
                        t_tiles[ft][:],
                        start=(ft == 0), stop=(ft == nft - 1))
                ys = y_acc[dt][:]
                if last:
                    # final add writes a compact fp16 staging tile, halving
                    # the output drain after the last matmul retires
                    y16 = yp.tile([128, W], f32r, tag=f"o{dt}",
                                  name=f"y16_{dt}")
                    nc.vector.tensor_add(y16[:], ys, pdt[:])
                    eng = nc.sync if dt % 2 == 0 else nc.scalar
                    eng.dma_start(yt[:, dt * W:(dt + 1) * W], y16[:])
                elif pg_idx == 0:
                    nc.vector.tensor_copy(ys, pdt[:])
                else:
                    nc.vector.tensor_add(ys, ys, pdt[:])

        prev = None  # (group idx, t_tiles, wd_t) of the previous f group
        fo = 0       # running f-column offset
        # down-chain schedule: assign the previous group's 8 d-tiles round-
        # robin across this group's up-chain slots
        for g, fg in enumerate(_GROUPS):
            ftg = fg // 128
            wg_t = wgp.tile([128, _ND * fg], f32r, tag=f"wg{fg}")
            wu_t = wup.tile([128, _ND * fg], f32r, tag=f"wu{fg}")
            wd_t = wdp.tile([128, ftg * D_MODEL], f32r, tag=f"wd{fg}")
            nc.sync.dma_start(wg_t[:], wg[:, _ND * fo:_ND * (fo + fg)])
            if g == 0:
                nc.sync.dma_start(x_t[:, 4 * W:], xt[:, 4 * W:])
            nc.sync.dma_start(wu_t[:], wu[:, _ND * fo:_ND * (fo + fg)])
            if prev is not None:
                # issue the PREVIOUS group's down weights now: they are
                # first consumed during THIS group's up phase, so delaying
                # them one group pulls every early gate/up arrival forward
                pfo, pftg, pwd_t = prev_wd
                nc.sync.dma_start(pwd_t[:], wd[:, (pfo // 128) * D_MODEL:
                                              (pfo // 128 + pftg) * D_MODEL])
            prev_wd = (fo, ftg, wd_t)

            # all gate chains (+ silu) first: the group's first compute
            # depends only on the gate descriptor, which arrives first
            g_acts = []
            for ft in range(ftg):
                psg = pg.tile([128, W], f32)
                for d in range(_ND):
                    nc.tensor.matmul(
                        psg[:],
                        wg_t[:, d * fg + ft * 128:d * fg + ft * 128 + 128],
                        x_sl(d),
                        start=(d == 0), stop=(d == _ND - 1))
                g_act = gap.tile([128, W], f32, tag=f"g{ft}")
                nc.scalar.activation(g_act[:], psg[:],
                                     mybir.ActivationFunctionType.Silu)
                g_acts.append(g_act)

            # up chains + swiglu muls, with the previous group's down
            # chains interleaved to spread PSUM/vector pressure
            t_tiles = []
            for ft in range(ftg):
                psu = pu.tile([128, W], f32)
                for d in range(_ND):
                    nc.tensor.matmul(
                        psu[:],
                        wu_t[:, d * fg + ft * 128:d * fg + ft * 128 + 128],
                        x_sl(d),
                        start=(d == 0), stop=(d == _ND - 1))
                t_t = tp.tile([128, W], f32r, tag=f"t{ft}")
                nc.vector.tensor_mul(t_t[:], g_acts[ft][:], psu[:])
                t_tiles.append(t_t)
                if prev is not None:
                    lo = _ND * ft // ftg
                    hi = _ND * (ft + 1) // ftg
                    emit_down(prev[0], prev[1], prev[2], range(lo, hi), False)
            prev = (g, t_tiles, wd_t)
            fo += fg
        pfo, pftg, pwd_t = prev_wd
        nc.sync.dma_start(pwd_t[:], wd[:, (pfo // 128) * D_MODEL:
                                      (pfo // 128 + pftg) * D_MODEL])
        emit_down(prev[0], prev[1], prev[2], range(_ND), True)

    nc.compile()
    return nc


def _pack_gu(w):
    # [D, F] -> [128, ND*F] in per-group blocks:
    # block_g[p, d*Fg + j] = w[d*128+p, fo_g + j]
    w = np.asarray(w).astype(_np_cdt()).reshape(_ND, 128, D_FF)
    blocks = []
    fo = 0
    for fg in _GROUPS:
        blk = w[:, :, fo:fo + fg]          # [ND, 128, Fg]
        blocks.append(blk.transpose(1, 0, 2).reshape(128, _ND * fg))
        fo += fg
    return np.ascontiguousarray(np.concatenate(blocks, axis=1))


def _pack_wd(w):
    # [F, D] -> [128, NF*D]: dram[p, c*D + dj] = w[c*128+p, dj]
    w = np.asarray(w).astype(_np_cdt())
    return np.ascontiguousarray(
        w.reshape(_NF, 128, D_MODEL).transpose(1, 0, 2).reshape(
            128, _NF * D_MODEL))


def _run_one(W, tok_lists, x_flat, packed_w, out_flat):
    from concourse.bass_utils import run_bass_kernel_spmd

    if W not in _nc_cache:
        _nc_cache[W] = _build_nc(W)
    nc = _nc_cache[W]

    D = x_flat.shape[1]
    in_maps = []
    for e in range(N_EXPERTS):
        toks = tok_lists[e]
        xt_e = np.zeros((D, W), dtype=_np_cdt())
        xt_e[:, :len(toks)] = x_flat[toks].T.astype(_np_cdt())
        # [D, W] -> [128, ND*W]: line p holds d-chunks side by side
        xt_e = np.ascontiguousarray(
            xt_e.reshape(_ND, 128, W).transpose(1, 0, 2).reshape(128, _ND * W))
        in_maps.append({
            "xt": xt_e,
            "wg": packed_w[e][0],
            "wu": packed_w[e][1],
            "wd": packed_w[e][2],
        })

    res = None
    for attempt in range(3):
        try:
            res = run_bass_kernel_spmd(nc, in_maps,
                                       core_ids=list(range(N_CORES)))
            break
        except Exception:
            if attempt == 2:
                raise
            import time
            time.sleep(3.0)
            try:
                import jax
                jax.clear_caches()
                jax.clear_backends()
            except Exception:
                pass
    for e in range(N_EXPERTS):
        toks = tok_lists[e]
        # yt: [128, ND*W] -> [ND*128, W] -> tokens
        y = res.results[e]["yt"].astype(np.float32).reshape(
            128, _ND, W).transpose(1, 0, 2).reshape(D, W)
        out_flat[toks] = y[:, :len(toks)].T


def kernel(x, expert_idx, w_gate, w_up, w_down):
    x = np.asarray(x, dtype=np.float32)
    idx = np.asarray(expert_idx).astype(np.int64)
    B, S, D = x.shape
    T = B * S
    x_flat = np.ascontiguousarray(x.reshape(T, D))
    idx_flat = idx.reshape(T)

    packed_w = [
        (_pack_gu(w_gate[e]), _pack_gu(w_up[e]), _pack_wd(w_down[e]))
        for e in range(N_EXPERTS)
    ]

    tok_lists = [np.nonzero(idx_flat == e)[0] for e in range(N_EXPERTS)]
    cap = max(1, max(len(t) for t in tok_lists))
    out_flat = np.zeros((T, D), dtype=np.float32)

    if cap <= 448:
        # normal path: one SPMD run, capacity = max expert load (floor 256
        # keeps DMA partition lines >= 512B)
        W = max(256, cap)
        _run_one(W, tok_lists, x_flat, packed_w, out_flat)
    else:
        # fallback for extreme routing imbalance: process tokens in
        # rounds of <=256 per expert, reusing one compiled W=256 program
        # (a known-good SBUF footprint)
        rounds = -(-cap // 256)
        for r in range(rounds):
            round_lists = [t[r * 256:(r + 1) * 256] for t in tok_lists]
            _run_one(256, round_lists, x_flat, packed_w, out_flat)

    return out_flat.reshape(B, S, D)


# revision 28
# speedup vs baseline: 1.1938x; 1.1938x over previous
"""Expert-parallel MoE SwiGLU kernel for 8 Trainium2 NeuronCores.

Strategy: expert parallelism with host-side dispatch/combine. Each of the
8 cores owns one expert's weights. The host routes tokens by expert_idx,
packs each expert's tokens as a transposed [D, W] panel (features on
partitions so no on-chip transposes are needed anywhere), and each core
runs a dense SwiGLU FFN:  yT = w_down.T-blocks @ (silu(wg.T@xT) * (wu.T@xT)).
Matmul operands stream as fp16 (fp32 PSUM accumulation; ~6e-4 max
relative error vs the fp32 reference), halving the weight traffic that
dominates this memory-bound kernel. fp8 (DoubleRow) was evaluated and
rejected: every quantization site alone (x, w_gate/up, t, w_down)
exceeds the 2e-2 max-relative-error budget (measured 2.8e-2..3.9e-2).

Schedule design, from perfetto-trace supply modeling:
- DMA rings are blocked until the ~7.2us framework preamble ends, then
  HBM sustains ~0.36MB/us/core. Descriptor issue costs ~650ns of
  ring-engine time, and a matmul chain needs its group's whole weight
  set anyway, so weights stream as ONE descriptor per matrix per
  f-group, all on the sync ring -- the only engine queue with no
  compute ops that could block its in-order FIFO (the scalar queue
  stalls behind silu ACTIVATEs waiting on PSUM).
- f-group widths are graduated (128x4, 256x2, 512x6): small groups in
  front so the first gate chain starts ~4us earlier than a uniform
  split allows, wide groups later to amortize issue cost.
- within a group, all gate chains+silus are emitted before up chains,
  matching the gate-before-up arrival order on the wire; the previous
  group's down-projection chains interleave into the up phase.
- each group's w_down descriptor is issued one group late (first use is
  one group later), pulling early gate/up arrivals forward.
- a ~17-matmul dummy burst at the start keeps the PE activity monitor
  fed so the clock is at 2.4GHz (not the cold 1.2GHz) when real work
  lands; without it the HAM revokes full clock for 7-13us.
- the final group's adds write fp16 staging tiles streamed out per
  d-tile on alternating rings, so the output drain after the last
  matmul is ~1us.
- all SBUF tiles are 2D with flat index arithmetic (3D tile slices
  measured ~27ns/matmul slower, though part of that may have been a
  throttled run); tile pools use bufs>=3 -- bufs=2 produced wrong
  results with consecutive small groups (tile lifetime spans 2 groups).

Steady state: matmuls issue every ~121ns (274-col moving panel = 114ns
at 2.4GHz + ~7ns sequencer overhead), PE-bound at the fp16 rate with
zero mid-stream stalls; HW exec ~112us vs the 91us pure-matmul floor
(preamble ~7us, supply-bound fill ~4.5us, per-instruction overhead
~5.5us, output drain + teardown ~4us).
"""

import numpy as np
from contextlib import ExitStack

D_MODEL = 1024
D_FF = 4096
N_EXPERTS = 8
N_CORES = 8

_ND = D_MODEL // 128  # 8 contraction chunks over d_model
_NF = D_FF // 128     # 32 f chunks

# graduated f-group widths (columns); must sum to D_FF
_GROUPS = [128, 128, 128, 128, 256, 256, 512, 512, 512, 512, 512, 512]
assert sum(_GROUPS) == D_FF

_nc_cache = {}

# compute dtype for matmul operands: "float32r" (safest), "float16", "bfloat16"
import os as _os
_CDT = _os.environ.get("MOE_KERNEL_DTYPE", "float16")
_WARM = int(_os.environ.get("MOE_WARMUP", "17"))


def _np_cdt():
    if _CDT == "float16":
        return np.float16
    if _CDT == "bfloat16":
        import ml_dtypes
        return ml_dtypes.bfloat16
    return np.float32


def _build_nc(W: int):
    """Build + schedule the per-core Bass program for token capacity W."""
    import concourse.bacc as bacc
    import concourse.tile as tile
    from concourse import mybir

    f32 = mybir.dt.float32
    f32r = getattr(mybir.dt, _CDT)

    nc = bacc.Bacc("TRN2", target_bir_lowering=False, debug=False,
                   num_devices=N_CORES)
    xt = nc.dram_tensor("xt", [128, _ND * W], f32r, kind="ExternalInput").ap()
    # wg/wu: per-group blocks [128, ND*Fg] concatenated along the free dim
    wg = nc.dram_tensor("wg", [128, _ND * D_FF], f32r,
                        kind="ExternalInput").ap()
    wu = nc.dram_tensor("wu", [128, _ND * D_FF], f32r,
                        kind="ExternalInput").ap()
    # wd: f-tile-major [128, NF*D]: line p holds rows (c*128+p) of w_down
    wd = nc.dram_tensor("wd", [128, _NF * D_MODEL], f32r,
                        kind="ExternalInput").ap()
    yt = nc.dram_tensor("yt", [128, _ND * W], f32r, kind="ExternalOutput").ap()

    with tile.TileContext(nc) as tc, ExitStack() as ctx:
        xpool = ctx.enter_context(tc.tile_pool(name="x", bufs=1))
        wgp = ctx.enter_context(tc.tile_pool(name="wgp", bufs=3))
        wup = ctx.enter_context(tc.tile_pool(name="wup", bufs=3))
        wdp = ctx.enter_context(tc.tile_pool(name="wdp", bufs=3))
        tp = ctx.enter_context(tc.tile_pool(name="tp", bufs=3))
        gap = ctx.enter_context(tc.tile_pool(name="gap", bufs=3))
        yp = ctx.enter_context(tc.tile_pool(name="yp", bufs=1))
        pg = ctx.enter_context(tc.tile_pool(name="pg", bufs=2, space="PSUM"))
        pu = ctx.enter_context(tc.tile_pool(name="pu", bufs=2, space="PSUM"))
        pd = ctx.enter_context(tc.tile_pool(name="pd", bufs=4, space="PSUM"))

        # Input panel: first descriptor on the sync ring.
        x_t = xpool.tile([128, _ND * W], f32r, tag="x")
        nc.sync.dma_start(x_t[:], xt[:])

        def x_sl(d):
            return x_t[:, d * W:d * W + W]

        y_acc = [yp.tile([128, W], f32, tag=f"y{d}", name=f"y_acc{d}")
                 for d in range(_ND)]

        scr_w = xpool.tile([128, 128], f32r, tag="scrw", name="scr_w")
        scr_x = xpool.tile([128, W], f32r, tag="scrx", name="scr_x")
        nc.vector.memset(scr_w[:], 0.0)
        nc.vector.memset(scr_x[:], 0.0)
        scr_p = pd.tile([128, W], f32, tag="pd", name="scr_p")
        scr_p2 = pd.tile([128, W], f32, tag="pd", name="scr_p2")
        _scr = [scr_p, scr_p2]

        def emit_warmup(n):
            for i in range(n):
                nc.tensor.matmul(_scr[i % 2][:], scr_w[:], scr_x[:],
                                 start=True, stop=True)

        # opening burst: continuous PE activity bridging the gap between
        # ring unblock and the first weight group's arrival
        emit_warmup(_WARM)

        def emit_down(pg_idx, t_tiles, wd_t, dts, last):
            # y[dt] += wd[fgroup rows, dt cols].T @ t   for dt in dts
            nft = len(t_tiles)
            for dt in dts:
                pdt = pd.tile([128, W], f32, tag="pd", name=f"pd_{pg_idx}_{dt}")
                for ft in range(nft):
                    nc.tensor.matmul(
                        pdt[:],
                        wd_t[:, ft * D_MODEL + dt * 128:
                             ft * D_MODEL + dt * 128 + 128],
                        t_tiles[ft][:],
                        start=(ft == 0), stop=(ft == nft - 1))
                ys = y_acc[dt][:]
                if last:
                    # final add writes a compact fp16 staging tile, halving
                    # the output drain after the last matmul retires
                    y16 = yp.tile([128, W], f32r, tag=f"o{dt}",
                                  name=f"y16_{dt}")
                    nc.vector.tensor_add(y16[:], ys, pdt[:])
                    eng = nc.sync if dt % 2 == 0 else nc.scalar
                    eng.dma_start(yt[:, dt * W:(dt + 1) * W], y16[:])
                elif pg_idx == 0:
                    nc.vector.tensor_copy(ys, pdt[:])
                else:
                    nc.vector.tensor_add(ys, ys, pdt[:])

        prev = None  # (group idx, t_tiles, wd_t) of the previous f group
        fo = 0       # running f-column offset
        # down-chain schedule: assign the previous group's 8 d-tiles round-
        # robin across this group's up-chain slots
        for g, fg in enumerate(_GROUPS):
            ftg = fg // 128
            wg_t = wgp.tile([128, _ND * fg], f32r, tag=f"wg{fg}")
            wu_t = wup.tile([128, _ND * fg], f32r, tag=f"wu{fg}")
            wd_t = wdp.tile([128, ftg * D_MODEL], f32r, tag=f"wd{fg}")
            nc.sync.dma_start(wg_t[:], wg[:, _ND * fo:_ND * (fo + fg)])
            nc.sync.dma_start(wu_t[:], wu[:, _ND * fo:_ND * (fo + fg)])
            if prev is not None:
                # issue the PREVIOUS group's down weights now: they are
                # first consumed during THIS group's up phase, so delaying
                # them one group pulls every early gate/up arrival forward
                pfo, pftg, pwd_t = prev_wd
                nc.sync.dma_start(pwd_t[:], wd[:, (pfo // 128) * D_MODEL:
                                              (pfo // 128 + pftg) * D_MODEL])
            prev_wd = (fo, ftg, wd_t)

            # all gate chains (+ silu) first: the group's first compute
            # depends only on the gate descriptor, which arrives first
            g_acts = []
            for ft in range(ftg):
                psg = pg.tile([128, W], f32)
                for d in range(_ND):
                    nc.tensor.matmul(
                        psg[:],
                        wg_t[:, d * fg + ft * 128:d * fg + ft * 128 + 128],
                        x_sl(d),
                        start=(d == 0), stop=(d == _ND - 1))
                g_act = gap.tile([128, W], f32, tag=f"g{ft}")
                nc.scalar.activation(g_act[:], psg[:],
                                     mybir.ActivationFunctionType.Silu)
                g_acts.append(g_act)

            # up chains + swiglu muls, with the previous group's down
            # chains interleaved to spread PSUM/vector pressure
            t_tiles = []
            for ft in range(ftg):
                psu = pu.tile([128, W], f32)
                for d in range(_ND):
                    nc.tensor.matmul(
                        psu[:],
                        wu_t[:, d * fg + ft * 128:d * fg + ft * 128 + 128],
                        x_sl(d),
                        start=(d == 0), stop=(d == _ND - 1))
                t_t = tp.tile([128, W], f32r, tag=f"t{ft}")
                nc.vector.tensor_mul(t_t[:], g_acts[ft][:], psu[:])
                t_tiles.append(t_t)
                if prev is not None:
                    lo = _ND * ft // ftg
                    hi = _ND * (ft + 1) // ftg
                    emit_down(prev[0], prev[1], prev[2], range(lo, hi), False)
            prev = (g, t_tiles, wd_t)
            fo += fg
        pfo, pftg, pwd_t = prev_wd
        nc.sync.dma_start(pwd_t[:], wd[:, (pfo // 128) * D_MODEL:
                                      (pfo // 128 + pftg) * D_MODEL])
        emit_down(prev[0], prev[1], prev[2], range(_ND), True)

    nc.compile()
    return nc


def _pack_gu(w):
    # [D, F] -> [128, ND*F] in per-group blocks:
    # block_g[p, d*Fg + j] = w[d*128+p, fo_g + j]
    w = np.asarray(w).astype(_np_cdt()).reshape(_ND, 128, D_FF)
    blocks = []
    fo = 0
    for fg in _GROUPS:
        blk = w[:, :, fo:fo + fg]          # [ND, 128, Fg]
        blocks.append(blk.transpose(1, 0, 2).reshape(128, _ND * fg))
        fo += fg
    return np.ascontiguousarray(np.concatenate(blocks, axis=1))


def _pack_wd(w):
    # [F, D] -> [128, NF*D]: dram[p, c*D + dj] = w[c*128+p, dj]
    w = np.asarray(w).astype(_np_cdt())
    return np.ascontiguousarray(
        w.reshape(_NF, 128, D_MODEL).transpose(1, 0, 2).reshape(
            128, _NF * D_MODEL))


def _run_one(W, tok_lists, x_flat, packed_w, out_flat):
    from concourse.bass_utils import run_bass_kernel_spmd

    if W not in _nc_cache:
        _nc_cache[W] = _build_nc(W)
    nc = _nc_cache[W]

    D = x_flat.shape[1]
    in_maps = []
    for e in range(N_EXPERTS):
        toks = tok_lists[e]
        xt_e = np.zeros((D, W), dtype=_np_cdt())
        xt_e[:, :len(toks)] = x_flat[toks].T.astype(_np_cdt())
        # [D, W] -> [128, ND*W]: line p holds d-chunks side by side
        xt_e = np.ascontiguousarray(
            xt_e.reshape(_ND, 128, W).transpose(1, 0, 2).reshape(128, _ND * W))
        in_maps.append({
            "xt": xt_e,
            "wg": packed_w[e][0],
            "wu": packed_w[e][1],
            "wd": packed_w[e][2],
        })

    res = None
    for attempt in range(3):
        try:
            res = run_bass_kernel_spmd(nc, in_maps,
                                       core_ids=list(range(N_CORES)))
            break
        except Exception:
            if attempt == 2:
                raise
            import time
            time.sleep(3.0)
            try:
                import jax
                jax.clear_caches()
                jax.clear_backends()
            except Exception:
                pass
    for e in range(N_EXPERTS):
        toks = tok_lists[e]
        # yt: [128, ND*W] -> [ND*128, W] -> tokens
        y = res.results[e]["yt"].astype(np.float32).reshape(
            128, _ND, W).transpose(1, 0, 2).reshape(D, W)
        out_flat[toks] = y[:, :len(toks)].T


def kernel(x, expert_idx, w_gate, w_up, w_down):
    x = np.asarray(x, dtype=np.float32)
    idx = np.asarray(expert_idx).astype(np.int64)
    B, S, D = x.shape
    T = B * S
    x_flat = np.ascontiguousarray(x.reshape(T, D))
    idx_flat = idx.reshape(T)

    packed_w = [
        (_pack_gu(w_gate[e]), _pack_gu(w_up[e]), _pack_wd(w_down[e]))
        for e in range(N_EXPERTS)
    ]

    tok_lists = [np.nonzero(idx_flat == e)[0] for e in range(N_EXPERTS)]
    cap = max(1, max(len(t) for t in tok_lists))
    out_flat = np.zeros((T, D), dtype=np.float32)

    if cap <= 448:
        # normal path: one SPMD run, capacity = max expert load (floor 256
        # keeps DMA partition lines >= 512B)
        W = max(256, cap)
        _run_one(W, tok_lists, x_flat, packed_w, out_flat)
    else:
        # fallback for extreme routing imbalance: process tokens in
        # rounds of <=256 per expert, reusing one compiled W=256 program
        # (a known-good SBUF footprint)
        rounds = -(-cap // 256)
        for r in range(rounds):
            round_lists = [t[r * 256:(r + 1) * 256] for t in tok_lists]
            _run_one(256, round_lists, x_flat, packed_w, out_flat)

    return out_flat.reshape(B, S, D)
